# revision 1
# baseline (speedup 1.0000x reference)
"""Trainium2 Bass kernel for nn_Block_32762010534337 (dense transformer block).

Strategy: stride-4 interleaved sequence parallel over 8 cores. Core c owns
tokens {4i + g} (g = c%4) of batch c//4 -- every core then has an IDENTICAL
causal attention program (q-tile qt needs exactly 8*(qt+1) key tiles); all
causality lives in per-core mask/index data. K/V are projected for the core's
own 512 tokens, AllGathered within the 4-core batch group (overlapped with Q
projection), and re-tiled into global key tiles via strided APs (K) and
partition-scatter DMAs (V). Scores are computed per 256-query tile into 4-tile
PSUM slabs, exp'd in one ScalarE ACTIVATE per slab, softmax denominators
accumulate on the PE (separate bank), and 1/x is computed as exp(-ln(x)) on
rows only. Activations stay feature-major end-to-end: the MLP proj output is
produced feature-major (no PE transposes) and the host transposes/scatters the
final [C, 512] tiles for free.
"""
import sys
import os

if "/opt/trn_rl_repo" not in sys.path:
    sys.path.insert(0, "/opt/trn_rl_repo")

import numpy as np

B, T, C = 2, 2048, 2048
NH, NKV, HD = 16, 4, 128
DFF = 4 * C
TQ = 512          # tokens per core (stride-4 interleaved)
CH = 256          # query tile
NT = C // 128     # 16 feature tiles
NF = DFF // 128   # 64 ff tiles
EPS = 1.1920929e-07
NCORES = 8

_CACHE = None


def _build():
    import concourse.bass as bass
    import concourse.tile as tile
    from concourse import mybir, bacc

    dt = mybir.dt
    f32, bf16 = dt.float32, dt.bfloat16
    Alu = mybir.AluOpType
    Act = mybir.ActivationFunctionType

    nc = bacc.Bacc("TRN2", target_bir_lowering=False, debug=False, num_devices=NCORES)

    for val in (EPS, HD * EPS):
        tns = nc.alloc_sbuf_tensor(f"const-f32-{val}", [128, 1], f32)
        nc.gpsimd.memset(tns.ap(), val)
        nc.const_aps.aps[(f32, val)] = tns.ap()
    nc.all_engine_barrier()

    xT = nc.declare_dram_parameter("xT", [C, TQ], bf16, isOutput=False)
    csc = nc.declare_dram_parameter("csc", [128, TQ], bf16, isOutput=False)
    css = nc.declare_dram_parameter("css", [128, TQ], bf16, isOutput=False)
    mask = nc.declare_dram_parameter("mask", [128, 8 * CH], bf16, isOutput=False)
    # weights host-pretiled (see _prep_weights)
    wq = nc.declare_dram_parameter("wq", [128, 4 * NT * TQ], bf16, isOutput=False)
    wk = nc.declare_dram_parameter("wk", [128, NT * TQ], bf16, isOutput=False)
    wv = nc.declare_dram_parameter("wv", [128, NT * TQ], bf16, isOutput=False)
    wo = nc.declare_dram_parameter("wo", [128, 4 * NT * TQ], bf16, isOutput=False)
    wfc = nc.declare_dram_parameter("wfc", [128, 16 * NT * TQ], bf16, isOutput=False)
    wproj = nc.declare_dram_parameter("wproj", [128, NT * NF * 128], bf16,
                                      isOutput=False)
    out_fm = nc.declare_dram_parameter("out", [C, TQ], f32, isOutput=True)

    ck_in = nc.dram_tensor("ck_in", [512, TQ], bf16)
    ck_out = nc.dram_tensor("ck_out", [2048, TQ], bf16)
    cv_in = nc.dram_tensor("cv_in", [512, TQ], bf16)
    cv_out = nc.dram_tensor("cv_out", [2048, TQ], bf16)

    with tile.TileContext(nc, num_cores=NCORES) as tc:
        with (
            tc.tile_pool(name="const", bufs=1) as constp,
            tc.tile_pool(name="persist", bufs=1) as pp,
            tc.tile_pool(name="work", bufs=3) as wpool,
            tc.tile_pool(name="wstream", bufs=3) as wsp,
        ):
            ones = constp.tile([128, 1], bf16, tag="ones")
            nc.gpsimd.memset(ones, 1.0)
            onesf = constp.tile([128, 1], f32, tag="onesf")
            nc.gpsimd.memset(onesf, 1.0)

            # x_mid^T lives across attention + MLP
            xmT = pp.tile([128, NT, TQ], f32, tag="xmT")

            def norm_row(ssq_ps, scale, bias, n, nb=128):
                """[1,n] psum sum-of-squares -> [nb,n] f32 bcast of
                (scale*x+bias)^(-1/2), via exp(-0.5*ln(.))."""
                ln = wpool.tile([1, n], f32, tag="srow", bufs=3, name="lnrow")
                nc.scalar.activation(ln[:], ssq_ps[:], Act.Ln, bias=bias,
                                     scale=scale)
                rs = wpool.tile([1, n], f32, tag="srow", bufs=3, name="rsrow")
                nc.scalar.activation(rs[:], ln[:], Act.Exp, scale=-0.5)
                sb = wpool.tile([nb, n], f32, tag="sbcast", bufs=3)
                nc.gpsimd.partition_broadcast(sb[:], rs[:])
                return sb

            def wslab2(param, base, width, n_i, name):
                """Stream [128, n_i, width] weights as two 8KB half-slabs."""
                half = n_i // 2
                tiles = []
                for hh in range(2):
                    ts = wsp.tile([128, half, width], bf16, tag="wslab",
                                  bufs=3, name=f"{name}_{hh}")
                    o = base + half * width * hh
                    nc.sync.dma_start(
                        ts[:],
                        param[:, o:o + half * width].rearrange(
                            "p (g t) -> p g t", t=width))
                    tiles.append(ts)

                def get(i, c0=None, c1=None):
                    t, j = tiles[i // half], i % half
                    return t[:, j] if c0 is None else t[:, j, c0:c1]
                return get

            with tc.tile_pool(name="main", bufs=1) as mp:
                csc_sb = mp.tile([128, TQ], bf16, tag="csc")
                nc.sync.dma_start(csc_sb[:], csc[:])
                css_sb = mp.tile([128, TQ], bf16, tag="css")
                nc.sync.dma_start(css_sb[:], css[:])
                mask_sb = mp.tile([128, 8, CH], bf16, tag="mask_sb")
                nc.sync.dma_start(mask_sb[:],
                                  mask.rearrange("p (d q) -> p d q", q=CH))

                xin = mp.tile([128, NT, TQ], bf16, tag="xin")
                qs_sb = mp.tile([128, NH, TQ], bf16, tag="qs_sb")
                k_sb = mp.tile([128, 4, NKV, TQ], bf16, tag="k_sb")
                v_sb = mp.tile([128, 4, 4, NKV * HD], bf16, tag="v_sb")
                yT = mp.tile([128, NH, TQ], bf16, tag="yT")

                def rope(ps, pool):
                    """psum [128,TQ] f32 -> rope'd bf16 sbuf tile."""
                    raw = pool.tile([128, TQ], bf16, tag="rraw", bufs=2,
                                    name="rraw")
                    nc.scalar.copy(raw[:], ps[:])
                    sw = pool.tile([128, TQ], bf16, tag="rsw", bufs=2,
                                   name="rsw")
                    nc.sync.dma_start(sw[0:64, :], raw[64:128, :])
                    nc.sync.dma_start(sw[64:128, :], raw[0:64, :])
                    rr = pool.tile([128, TQ], bf16, tag="rr", bufs=6,
                                   name="rr")
                    nc.vector.tensor_tensor(rr[:], raw[:], csc_sb[:], Alu.mult)
                    t2 = pool.tile([128, TQ], bf16, tag="rt2", bufs=2,
                                   name="rt2")
                    nc.vector.tensor_tensor(t2[:], sw[:], css_sb[:], Alu.mult)
                    nc.vector.tensor_tensor(rr[:], rr[:], t2[:], Alu.add)
                    return rr

                def sumsq(rr, pool):
                    sq = pool.tile([128, TQ], bf16, tag="rsq", bufs=6,
                                   name="rsq")
                    nc.vector.tensor_tensor(sq[:], rr[:], rr[:], Alu.mult)
                    return sq

                with tc.tile_pool(name="psA", bufs=1, space="PSUM") as psA:
                    # x loaded once; Q/K project RAW x (their head-rmsnorm
                    # cancels the per-token pre-norm scale exactly)
                    for quad in range(4):
                        nc.sync.dma_start(
                            xin[:, 4 * quad:4 * (quad + 1), :],
                            xT[512 * quad:512 * (quad + 1), :].rearrange(
                                "(i p) t -> p i t", p=128))

                    # ---- K heads first: project + rope + packed k-norm ----
                    kps = [psA.tile([128, TQ], f32, tag="qkv", bufs=4,
                                    name=f"kps_{_k}") for _k in range(4)]
                    wkf = wslab2(wk, 0, TQ, NT, "wk")
                    for k in range(4):
                        for i in range(NT):
                            nc.tensor.matmul(kps[k][:],
                                             lhsT=wkf(i, 128 * k, 128 * (k + 1)),
                                             rhs=xin[:, i],
                                             start=(i == 0), stop=(i == NT - 1))

                    # pre-attention rmsnorm row (for V scaling only)
                    ssq_ps = psA.tile([1, TQ], f32, tag="row", bufs=3)
                    for i in range(NT):
                        xsq = wpool.tile([128, TQ], bf16, tag="xsq", bufs=6)
                        nc.vector.tensor_tensor(xsq[:], xin[:, i], xin[:, i],
                                                Alu.mult)
                        nc.tensor.matmul(ssq_ps[:], lhsT=ones[:], rhs=xsq[:],
                                         start=(i == 0), stop=(i == NT - 1))
                    s1ln = wpool.tile([1, TQ], f32, tag="srow", bufs=3,
                                      name="s1ln")
                    nc.scalar.activation(s1ln[:], ssq_ps[:], Act.Ln, bias=EPS,
                                         scale=1.0 / C)
                    s1rs = wpool.tile([1, TQ], f32, tag="srow", bufs=3,
                                      name="s1rs")
                    nc.scalar.activation(s1rs[:], s1ln[:], Act.Exp, scale=-0.5)
                    # transpose the rsqrt row to per-token columns (PE)
                    s1c = psA.tile([128, 4], f32, tag="scol", bufs=1)
                    for t in range(4):
                        nc.tensor.transpose(s1c[:, t:t + 1],
                                            s1rs[0:1, 128 * t:128 * (t + 1)],
                                            onesf[0:1, 0:1])

                    kpk = wpool.tile([4, TQ], f32, tag="srow4", bufs=2,
                                     name="kpk")
                    rrs_k = []
                    for kh in range(4):
                        rr = rope(kps[kh], mp)
                        sq = sumsq(rr, mp)
                        sps = psA.tile([1, TQ], f32, tag="row", bufs=3)
                        nc.tensor.matmul(sps[:], lhsT=ones[:], rhs=sq[:],
                                         start=True, stop=True)
                        srow = wpool.tile([1, TQ], f32, tag="srow", bufs=3,
                                          name=f"ksr{kh}")
                        nc.vector.tensor_copy(out=srow[:], in_=sps[:])
                        nc.sync.dma_start(kpk[kh:kh + 1, :], srow[:])
                        rrs_k.append(rr)
                    kln = wpool.tile([4, TQ], f32, tag="srow4", bufs=2,
                                     name="kln")
                    nc.scalar.activation(kln[:], kpk[:], Act.Ln, bias=EPS,
                                         scale=1.0 / HD)
                    krs = wpool.tile([4, TQ], f32, tag="srow4", bufs=2,
                                     name="krs")
                    nc.scalar.activation(krs[:], kln[:], Act.Exp, scale=-0.5)
                    for kh in range(4):
                        rowt = wpool.tile([1, TQ], f32, tag="srow", bufs=3,
                                          name=f"kurow{kh}")
                        nc.sync.dma_start(rowt[:], krs[kh:kh + 1, :])
                        sb = wpool.tile([128, TQ], f32, tag="sbcast", bufs=3)
                        nc.gpsimd.partition_broadcast(sb[:], rowt[:])
                        kt = mp.tile([128, TQ], bf16, tag="ktile", bufs=2,
                                     name="kt")
                        nc.vector.tensor_tensor(kt[:], rrs_k[kh][:], sb[:],
                                                Alu.mult)
                        nc.sync.dma_start(ck_in[128 * kh:128 * (kh + 1), :],
                                          kt[:])
                    nc.gpsimd.collective_compute(
                        "AllGather", Alu.bypass,
                        replica_groups=[[0, 1, 2, 3], [4, 5, 6, 7]],
                        ins=[ck_in[:]], outs=[ck_out[:]])

                    # ---- V heads: project raw x token-major, then apply the
                    # pre-norm scale per token (psum column) ----
                    wvf = wslab2(wv, 0, TQ, NT, "wv")
                    for t in range(4):
                        vps = psA.tile([128, TQ], f32, tag="qkv", bufs=4,
                                       name=f"vps_{t}")
                        for i in range(NT):
                            nc.tensor.matmul(vps[:],
                                             lhsT=xin[:, i, 128 * t:128 * (t + 1)],
                                             rhs=wvf(i),
                                             start=(i == 0), stop=(i == NT - 1))
                        vb = wpool.tile([128, TQ], bf16, tag="vb", bufs=2)
                        nc.vector.tensor_scalar_mul(vb[:], vps[:], s1c[:, t:t + 1])
                        nc.sync.dma_start(cv_in[128 * t:128 * (t + 1), :],
                                          vb[:])
                    nc.gpsimd.collective_compute(
                        "AllGather", Alu.bypass,
                        replica_groups=[[0, 1, 2, 3], [4, 5, 6, 7]],
                        ins=[cv_in[:]], outs=[cv_out[:]])

                    # ---- Q heads: project raw x + rope + deferred q-norm ----
                    pending = None

                    def finish_q(pend):
                        hg, rrs, sqs = pend
                        qpk = wpool.tile([4, TQ], f32, tag="srow4", bufs=2,
                                         name=f"qpk{hg}")
                        for k in range(4):
                            sps = psA.tile([1, TQ], f32, tag="row", bufs=3)
                            nc.tensor.matmul(sps[:], lhsT=ones[:],
                                             rhs=sqs[k][:],
                                             start=True, stop=True)
                            srow = wpool.tile([1, TQ], f32, tag="srow",
                                              bufs=3, name=f"qsr{hg}_{k}")
                            nc.vector.tensor_copy(out=srow[:], in_=sps[:])
                            nc.sync.dma_start(qpk[k:k + 1, :], srow[:])
                        qln = wpool.tile([4, TQ], f32, tag="srow4", bufs=2,
                                         name=f"qln{hg}")
                        nc.scalar.activation(qln[:], qpk[:], Act.Ln,
                                             bias=HD * EPS, scale=1.0)
                        qrs = wpool.tile([4, TQ], f32, tag="srow4", bufs=2,
                                         name=f"qrs{hg}")
                        nc.scalar.activation(qrs[:], qln[:], Act.Exp,
                                             scale=-0.5)
                        for k in range(4):
                            rowt = wpool.tile([1, TQ], f32, tag="srow",
                                              bufs=3, name=f"qur{hg}_{k}")
                            nc.sync.dma_start(rowt[:], qrs[k:k + 1, :])
                            sb = wpool.tile([128, TQ], f32, tag="sbcast",
                                            bufs=3)
                            nc.gpsimd.partition_broadcast(sb[:], rowt[:])
                            nc.vector.tensor_tensor(qs_sb[:, 4 * hg + k],
                                                    rrs[k][:], sb[:], Alu.mult)

                    for hg in range(4):
                        qps = [psA.tile([128, TQ], f32, tag="qkv", bufs=4,
                                        name=f"qps{hg}_{_k}") for _k in range(4)]
                        wqf = wslab2(wq, NT * TQ * hg, TQ, NT, f"wq{hg}")
                        rrs = []
                        sqs = []
                        for k in range(4):
                            for i in range(NT):
                                nc.tensor.matmul(qps[k][:],
                                                 lhsT=wqf(i, 128 * k, 128 * (k + 1)),
                                                 rhs=xin[:, i],
                                                 start=(i == 0),
                                                 stop=(i == NT - 1))
                            rrs.append(rope(qps[k], mp))
                            sqs.append(sumsq(rrs[k], mp))
                        if pending is not None:
                            finish_q(pending)
                        pending = (hg, rrs, sqs)
                    finish_q(pending)

                # ---- load gathered K/V (key tile m = rank m%4, its
                # column block m//4; contiguous in both buffers) ----
                for gp in range(4):
                    nc.sync.dma_start(
                        k_sb[:, gp],
                        ck_out[512 * gp:512 * (gp + 1), :].rearrange(
                            "(kh p) t -> p kh t", p=128))
                    nc.sync.dma_start(
                        v_sb[:, gp],
                        cv_out[512 * gp:512 * (gp + 1), :].rearrange(
                            "(cb p) f -> p cb f", p=128))

                # ---- attention (unnormalized y; den batched) ----
                den_all = mp.tile([32, CH], f32, tag="den_all")
                with tc.tile_pool(name="psB", bufs=1, space="PSUM") as psB:
                    for kh in range(NKV):
                        for j in range(4):
                            h = 4 * kh + j
                            for qt in range(2):
                                u = 2 * h + qt
                                nk = 8 * (qt + 1)
                                dlo = 8 * qt
                                y_ps = psB.tile([128, CH], f32, tag="y",
                                                bufs=2)
                                den_ps = psB.tile([1, CH], f32, tag="den",
                                                  bufs=2)
                                for grp in range(nk // 4):
                                    sc = psB.tile([128, 4, CH], f32, tag="sc",
                                                  bufs=2)
                                    p_sb = mp.tile([128, 4, CH], bf16,
                                                   tag="p_sb", bufs=3)
                                    for mi in range(4):
                                        m = 4 * grp + mi
                                        gp, cb = m % 4, m // 4
                                        nc.tensor.matmul(
                                            sc[:, mi],
                                            lhsT=k_sb[:, gp, kh,
                                                      128 * cb:128 * (cb + 1)],
                                            rhs=qs_sb[:, h,
                                                      CH * qt:CH * (qt + 1)],
                                            start=True, stop=True)
                                    nc.scalar.activation(p_sb[:], sc[:],
                                                         Act.Exp)
                                    for mi in range(4):
                                        m = 4 * grp + mi
                                        gp, cb = m % 4, m // 4
                                        if m >= dlo:
                                            nc.vector.tensor_tensor(
                                                p_sb[:, mi], p_sb[:, mi],
                                                mask_sb[:, m - dlo], Alu.mult)
                                        nc.tensor.matmul(
                                            y_ps[:],
                                            lhsT=v_sb[:, gp, cb,
                                                      128 * kh:128 * (kh + 1)],
                                            rhs=p_sb[:, mi],
                                            start=(m == 0), stop=(m == nk - 1))
                                        nc.tensor.matmul(
                                            den_ps[:],
                                            lhsT=ones[:], rhs=p_sb[:, mi],
                                            start=(m == 0), stop=(m == nk - 1))
                                nc.vector.tensor_copy(
                                    out=yT[:, h, CH * qt:CH * (qt + 1)],
                                    in_=y_ps[:])
                                drow = wpool.tile([1, CH], f32, tag="srow",
                                                  bufs=3, name="drow")
                                nc.vector.tensor_copy(out=drow[:],
                                                      in_=den_ps[:])
                                nc.sync.dma_start(den_all[u:u + 1, :],
                                                  drow[:])
                    # one Ln+Exp for all 32 denominators, then scale yT
                    rcl = wpool.tile([32, CH], f32, tag="srow32", bufs=2,
                                     name="rcl")
                    nc.scalar.activation(rcl[:], den_all[:], Act.Ln)
                    rca = wpool.tile([32, CH], f32, tag="srow32", bufs=2,
                                     name="rca")
                    nc.scalar.activation(rca[:], rcl[:], Act.Exp, scale=-1.0)
                    for h in range(NH):
                        for qt in range(2):
                            u = 2 * h + qt
                            rowt = wpool.tile([1, CH], f32, tag="srow",
                                              bufs=3, name="durow")
                            nc.sync.dma_start(rowt[:], rca[u:u + 1, :])
                            db = wpool.tile([128, CH], f32, tag="dbcast",
                                            bufs=2)
                            nc.gpsimd.partition_broadcast(db[:], rowt[:])
                            nc.vector.tensor_tensor(
                                yT[:, h, CH * qt:CH * (qt + 1)],
                                yT[:, h, CH * qt:CH * (qt + 1)],
                                db[:], Alu.mult)

                # ---- wo projection + residual (feature-major xmT) ----
                with tc.tile_pool(name="psC", bufs=1, space="PSUM") as psC:
                    for n4 in range(4):
                        wof = wslab2(wo, NT * TQ * n4, TQ, NT, f"wo{n4}")
                        for k in range(4):
                            att_ps = psC.tile([128, TQ], f32, tag="att",
                                              bufs=4)
                            for hh in range(NH):
                                nc.tensor.matmul(
                                    att_ps[:],
                                    lhsT=wof(hh, 128 * k, 128 * (k + 1)),
                                    rhs=yT[:, hh, :],
                                    start=(hh == 0), stop=(hh == NH - 1))
                            n = 4 * n4 + k
                            nc.vector.tensor_tensor(xmT[:, n], att_ps[:],
                                                    xin[:, n], Alu.add)
            # main pool closed (frees attention SBUF)

            # ---- MLP ----
            with tc.tile_pool(name="mlp", bufs=1) as mlpp:
                h2T = mlpp.tile([128, NT, TQ], bf16, tag="h2T")
                a_sb = mlpp.tile([128, NF, TQ], bf16, tag="a_sb")

                with tc.tile_pool(name="psC2", bufs=1, space="PSUM") as psC2:
                    ssq2 = psC2.tile([1, TQ], f32, tag="row", bufs=2)
                    for i in range(NT):
                        xsq = wpool.tile([128, TQ], bf16, tag="xsq", bufs=6)
                        nc.vector.tensor_tensor(xsq[:], xmT[:, i], xmT[:, i],
                                                Alu.mult)
                        nc.tensor.matmul(ssq2[:], lhsT=ones[:], rhs=xsq[:],
                                         start=(i == 0), stop=(i == NT - 1))
                    s2b = norm_row(ssq2, 1.0 / C, EPS, TQ)
                    for i in range(NT):
                        nc.vector.tensor_tensor(h2T[:, i], xmT[:, i], s2b[:],
                                                Alu.mult)

                # fc + relu^2 (feature-major a)
                with tc.tile_pool(name="psD", bufs=1, space="PSUM") as psD:
                    for jc in range(16):
                        wfcf = wslab2(wfc, NT * TQ * jc, TQ, NT, f"wfc{jc}")
                        for jf in range(4):
                            f_ps = psD.tile([128, TQ], f32, tag="f", bufs=6)
                            for i in range(NT):
                                nc.tensor.matmul(
                                    f_ps[:],
                                    lhsT=wfcf(i, 128 * jf, 128 * (jf + 1)),
                                    rhs=h2T[:, i],
                                    start=(i == 0), stop=(i == NT - 1))
                            f = 4 * jc + jf
                            r_bf = wpool.tile([128, TQ], bf16, tag="r_bf")
                            nc.scalar.activation(r_bf[:], f_ps[:], Act.Relu)
                            nc.vector.tensor_tensor(a_sb[:, f], r_bf[:],
                                                    r_bf[:], Alu.mult)

                # proj: weight-stationary, feature-major output + residual
                with tc.tile_pool(name="psE", bufs=1, space="PSUM") as psE:
                    for n in range(16):
                        wpf = wslab2(wproj, NF * 128 * n, 128, NF, f"wp{n}")
                        o_ps = psE.tile([128, TQ], f32, tag="o", bufs=3)
                        for f in range(NF):
                            nc.tensor.matmul(o_ps[:], lhsT=wpf(f),
                                             rhs=a_sb[:, f, :],
                                             start=(f == 0),
                                             stop=(f == NF - 1))
                        ov = wpool.tile([128, TQ], f32, tag="ov", bufs=3)
                        nc.vector.tensor_tensor(ov[:], o_ps[:], xmT[:, n],
                                                Alu.add)
                        nc.sync.dma_start(out_fm[128 * n:128 * (n + 1), :],
                                          ov[:])

    nc.compile()
    return nc


def _prep_weights(wq, wk, wv, wo, w_fc, w_proj):
    import ml_dtypes
    bf = ml_dtypes.bfloat16

    def tile_w(w, chunk):
        # [R, F] -> [128, (F//chunk) * (R//128) * chunk]
        R, F = w.shape
        t = w.reshape(R // 128, 128, F // chunk, chunk)
        t = t.transpose(1, 2, 0, 3)
        return np.ascontiguousarray(t.reshape(128, -1)).astype(bf)

    return {
        "wq": tile_w(np.asarray(wq, np.float32), TQ),
        "wk": tile_w(np.asarray(wk, np.float32), NKV * HD),
        "wv": tile_w(np.asarray(wv, np.float32), NKV * HD),
        "wo": tile_w(np.asarray(wo, np.float32), TQ),
        "wfc": tile_w(np.asarray(w_fc, np.float32), TQ),
        "wproj": tile_w(np.asarray(w_proj, np.float32), 128),
    }


def _make_in_maps(x, cos, sin, weights_b):
    import ml_dtypes
    bf = ml_dtypes.bfloat16
    cosT = cos[0, :, 0, :].T  # [64, T]
    sinT = sin[0, :, 0, :].T
    kk = np.arange(128)
    qq = np.arange(CH)
    dd = np.arange(8)                           # band tile: d = dd//4, gp = dd%4
    in_maps = []
    for c in range(NCORES):
        b, g = divmod(c, 4)
        idx = 4 * np.arange(TQ) + g             # own token positions
        # key pos = 512*(2qt+d) + 4k + gp ; query pos = 1024qt + 4q + g
        off = 128 * (dd // 4) + (dd % 4 > g)
        msk = (qq[None, None, :] - kk[:, None, None]
               >= off[None, :, None]).astype(np.float32)
        m = {
            "xT": np.ascontiguousarray(x[b, idx, :].T).astype(bf),
            "csc": np.ascontiguousarray(
                np.concatenate([cosT[:, idx], cosT[:, idx]],
                               axis=0)).astype(bf),
            "css": np.ascontiguousarray(
                np.concatenate([sinT[:, idx], -sinT[:, idx]],
                               axis=0)).astype(bf),
            "mask": np.ascontiguousarray(msk.reshape(128, 8 * CH)).astype(bf),
        }
        m.update(weights_b)
        in_maps.append(m)
    return in_maps


def kernel(x, cos, sin, wq, wk, wv, wo, w_fc, w_proj):
    global _CACHE
    from concourse.bass_utils import run_bass_kernel_spmd

    x = np.asarray(x, np.float32)
    cos = np.asarray(cos, np.float32)
    sin = np.asarray(sin, np.float32)
    weights_b = _prep_weights(wq, wk, wv, wo, w_fc, w_proj)

    if _CACHE is None:
        _CACHE = _build()
    nc = _CACHE

    in_maps = _make_in_maps(x, cos, sin, weights_b)
    res = run_bass_kernel_spmd(nc, in_maps, list(range(NCORES)))
    out = np.empty((B, T, C), np.float32)
    for c in range(NCORES):
        b, g = divmod(c, 4)
        idx = 4 * np.arange(TQ) + g
        out[b, idx, :] = res.results[c]["out"].T
    return out



# revision 4
# speedup vs baseline: 1.0859x; 1.0859x over previous
"""Trainium2 Bass kernel for nn_Block_32762010534337 (dense transformer block).

Strategy: stride-4 interleaved sequence parallel over 8 cores (core c owns
tokens {4i + c%4} of batch c//4); every core runs an identical causal
attention program. v2 additions over the bf16 baseline:

- QKV + wo projections run in fp8e4m3 with MatmulPerfMode.DoubleRow (two
  128-contraction subtiles per PE instruction). Weights are host-quantized
  at 64x scale; the rmsnorms are scale-invariant so Q/K dequantize for
  free, V folds 1/64 into its norm-row exp bias, and wo's 512x composite
  scale is removed in the PSUM evacuation.
- Attention: exp() writes fp8 p-tiles directly (score shift -ln16 keeps
  the range in fp8), the PV and denominator matmuls use DoubleRow over
  key-tile pairs, and the denominator reciprocal runs on the DVE
  (no scalar Ln/Exp -> no ACT table switches inside attention).
- K and V share one AllGather (one collective floor instead of two).
- x is host-pretiled to partition-major so the input loads are single
  contiguous 128-partition DMAs.

MLP stays bf16 (fp8 would blow the 2e-2 error budget through relu^2).
"""
import sys
import os

if "/opt/trn_rl_repo" not in sys.path:
    sys.path.insert(0, "/opt/trn_rl_repo")

import numpy as np

B, T, C = 2, 2048, 2048
NH, NKV, HD = 16, 4, 128
DFF = 4 * C
TQ = 512          # tokens per core (stride-4 interleaved)
CH = 256          # query tile
NT = C // 128     # 16 feature tiles
NF = DFF // 128   # 64 ff tiles
EPS = 1.1920929e-07
NCORES = 8
SW = 64.0                    # fp8 weight pre-scale
NLN16 = -2.772588722239781   # -ln 16: p8 = exp(s)/16
NLN64 = -4.1588830833596715  # -ln 64: v norm-row descale

_CACHE = None


def _build():
    import concourse.bass as bass
    import concourse.tile as tile
    from concourse import mybir, bacc

    dt = mybir.dt
    f32, bf16, fp8 = dt.float32, dt.bfloat16, dt.float8e4
    Alu = mybir.AluOpType
    Act = mybir.ActivationFunctionType
    DR = mybir.MatmulPerfMode.DoubleRow

    nc = bacc.Bacc("TRN2", target_bir_lowering=False, debug=False, num_devices=NCORES)

    for val in (EPS, HD * EPS, NLN16, NLN64):
        tns = nc.alloc_sbuf_tensor(f"const-f32-{val}", [128, 1], f32)
        nc.gpsimd.memset(tns.ap(), val)
        nc.const_aps.aps[(f32, val)] = tns.ap()
    nc.all_engine_barrier()

    # host-pretiled inputs ([128, i, t] partition-major)
    x8p = nc.declare_dram_parameter("x8p", [128, NT * TQ], fp8, isOutput=False)
    xp = nc.declare_dram_parameter("xp", [128, NT * TQ], bf16, isOutput=False)
    csc = nc.declare_dram_parameter("csc", [128, TQ], bf16, isOutput=False)
    css = nc.declare_dram_parameter("css", [128, TQ], bf16, isOutput=False)
    mask = nc.declare_dram_parameter("mask", [128, 8 * CH], bf16, isOutput=False)
    # weights host-pretiled (see _prep_weights); q/k/v/o fp8 at 64x
    wq = nc.declare_dram_parameter("wq", [128, 4 * NT * TQ], fp8, isOutput=False)
    wk = nc.declare_dram_parameter("wk", [128, NT * TQ], fp8, isOutput=False)
    wv = nc.declare_dram_parameter("wv", [128, NT * TQ], fp8, isOutput=False)
    wo = nc.declare_dram_parameter("wo", [128, 4 * NT * TQ], fp8, isOutput=False)
    wfc = nc.declare_dram_parameter("wfc", [128, 16 * NT * TQ], bf16, isOutput=False)
    wproj = nc.declare_dram_parameter("wproj", [128, NT * NF * 128], bf16,
                                      isOutput=False)
    out_fm = nc.declare_dram_parameter("out", [C, TQ], f32, isOutput=True)

    ckv_in = nc.dram_tensor("ckv_in", [1024, TQ], bf16)
    ckv_out = nc.dram_tensor("ckv_out", [4096, TQ], bf16)

    with tile.TileContext(nc, num_cores=NCORES) as tc:
        with (
            tc.tile_pool(name="const", bufs=1) as constp,
            tc.tile_pool(name="persist", bufs=1) as pp,
            tc.tile_pool(name="work", bufs=3) as wpool,
            tc.tile_pool(name="wstream", bufs=3) as wsp,
        ):
            ones = constp.tile([128, 1], bf16, tag="ones")
            nc.gpsimd.memset(ones, 1.0)
            onesf = constp.tile([128, 1], f32, tag="onesf")
            nc.gpsimd.memset(onesf, 1.0)
            # den lhsT: value 1/8 folds the y rescale (yT = 8*y) for free
            ones2 = constp.tile([128, 2, 32], fp8, tag="ones2")
            nc.gpsimd.memset(ones2, 0.125)

            # x_mid^T lives across attention + MLP
            xmT = pp.tile([128, NT, TQ], f32, tag="xmT")

            def norm_row(ssq_ps, scale, bias, n, nb=128):
                """[1,n] psum sum-of-squares -> [nb,n] f32 bcast of
                (scale*x+bias)^(-1/2), via exp(-0.5*ln(.))."""
                ln = wpool.tile([1, n], f32, tag="srow", bufs=3, name="lnrow")
                nc.scalar.activation(ln[:], ssq_ps[:], Act.Ln, bias=bias,
                                     scale=scale)
                rs = wpool.tile([1, n], f32, tag="srow", bufs=3, name="rsrow")
                nc.scalar.activation(rs[:], ln[:], Act.Exp, scale=-0.5)
                sb = wpool.tile([nb, n], f32, tag="sbcast", bufs=3)
                nc.gpsimd.partition_broadcast(sb[:], rs[:])
                return sb

            def wslab2(param, base, width, n_i, name, wdt):
                """Stream [128, n_i, width] weights as two half-slabs."""
                half = n_i // 2
                tiles = []
                for hh in range(2):
                    ts = wsp.tile([128, half, width], wdt, tag="wslab",
                                  bufs=3, name=f"{name}_{hh}")
                    o = base + half * width * hh
                    nc.sync.dma_start(
                        ts[:],
                        param[:, o:o + half * width].rearrange(
                            "p (g t) -> p g t", t=width))
                    tiles.append(ts)

                def get(i, c0=None, c1=None, pair=False):
                    t, j = tiles[i // half], i % half
                    sl = slice(j, j + 2) if pair else j
                    return t[:, sl] if c0 is None else t[:, sl, c0:c1]
                return get

            with tc.tile_pool(name="main", bufs=1) as mp:
                xin = mp.tile([128, NT, TQ], bf16, tag="xin")
                qs_sb = mp.tile([128, NH, TQ], bf16, tag="qs_sb")
                k_sb = mp.tile([128, 4, NKV, TQ], bf16, tag="k_sb")
                v8_sb = mp.tile([128, 4, 4, NKV * HD], fp8, tag="v8_sb")
                yT8 = mp.tile([128, NH, TQ], fp8, tag="yT8")
                qp_cm = tc.tile_pool(name="qkvtmp", bufs=1)
                qp = qp_cm.__enter__()
                x8 = qp.tile([128, NT, TQ], fp8, tag="x8")
                nc.sync.dma_start(
                    x8[:], x8p[:].rearrange("p (i t) -> p i t", t=TQ))
                wkf = wslab2(wk, 0, TQ, NT, "wk", fp8)
                nc.sync.dma_start(
                    xin[:], xp[:].rearrange("p (i t) -> p i t", t=TQ))
                csc_sb = qp.tile([128, TQ], bf16, tag="csc")
                nc.sync.dma_start(csc_sb[:], csc[:])
                css_sb = qp.tile([128, TQ], bf16, tag="css")
                nc.sync.dma_start(css_sb[:], css[:])

                def rope(ps, pool):
                    """psum [128,TQ] f32 -> rope'd bf16 sbuf tile."""
                    raw = pool.tile([128, TQ], bf16, tag="rraw", bufs=2,
                                    name="rraw")
                    nc.scalar.copy(raw[:], ps[:])
                    sw = pool.tile([128, TQ], bf16, tag="rsw", bufs=2,
                                   name="rsw")
                    nc.sync.dma_start(sw[0:64, :], raw[64:128, :])
                    nc.sync.dma_start(sw[64:128, :], raw[0:64, :])
                    rr = pool.tile([128, TQ], bf16, tag="rr", bufs=4,
                                   name="rr")
                    nc.vector.tensor_tensor(rr[:], raw[:], csc_sb[:], Alu.mult)
                    t2 = pool.tile([128, TQ], bf16, tag="rt2", bufs=2,
                                   name="rt2")
                    nc.vector.tensor_tensor(t2[:], sw[:], css_sb[:], Alu.mult)
                    nc.vector.tensor_tensor(rr[:], rr[:], t2[:], Alu.add)
                    return rr

                def sumsq(rr, pool):
                    sq = pool.tile([128, TQ], bf16, tag="rsq", bufs=4,
                                   name="rsq")
                    nc.vector.tensor_tensor(sq[:], rr[:], rr[:], Alu.mult)
                    return sq

                with tc.tile_pool(name="psA", bufs=1, space="PSUM") as psA:
                    # ---- K heads: project (fp8 DR) + rope + packed k-norm.
                    # Q/K project RAW x (head-rmsnorm cancels the pre-norm
                    # scale AND the 64x fp8 weight scale exactly).
                    kps = [psA.tile([128, TQ], f32, tag="qkv", bufs=4,
                                    name=f"kps_{_k}") for _k in range(4)]
                    for ip in range(0, NT, 2):
                        for k in range(4):
                            nc.tensor.matmul(kps[k][:],
                                             lhsT=wkf(ip, 128 * k,
                                                      128 * (k + 1), pair=True),
                                             rhs=x8[:, ip:ip + 2, :],
                                             start=(ip == 0),
                                             stop=(ip == NT - 2),
                                             perf_mode=DR)

                    # pre-attention rmsnorm row (for V scaling only)
                    ssq_ps = psA.tile([1, TQ], f32, tag="row", bufs=3)
                    for i in range(NT):
                        xsq = wpool.tile([128, TQ], bf16, tag="xsq", bufs=6)
                        nc.vector.tensor_tensor(xsq[:], xin[:, i], xin[:, i],
                                                Alu.mult)
                        nc.tensor.matmul(ssq_ps[:], lhsT=ones[:], rhs=xsq[:],
                                         start=(i == 0), stop=(i == NT - 1))
                    s1ln = wpool.tile([1, TQ], f32, tag="srow", bufs=3,
                                      name="s1ln")
                    nc.scalar.activation(s1ln[:], ssq_ps[:], Act.Ln, bias=EPS,
                                         scale=1.0 / C)
                    # extra -ln64 kills the 64x fp8 weight scale on V
                    s1rs = wpool.tile([1, TQ], f32, tag="srow", bufs=3,
                                      name="s1rs")
                    nc.scalar.activation(s1rs[:], s1ln[:], Act.Exp, scale=-0.5,
                                         bias=NLN64)
                    # transpose the rsqrt row to per-token columns (PE)
                    s1c = psA.tile([128, 4], f32, tag="scol", bufs=1)
                    for t in range(4):
                        nc.tensor.transpose(s1c[:, t:t + 1],
                                            s1rs[0:1, 128 * t:128 * (t + 1)],
                                            onesf[0:1, 0:1])

                    kpk = wpool.tile([4, TQ], f32, tag="srow4", bufs=2,
                                     name="kpk")
                    rrs_k = []
                    for kh in range(4):
                        rr = rope(kps[kh], qp)
                        sq = sumsq(rr, qp)
                        sps = psA.tile([1, TQ], f32, tag="row", bufs=3)
                        nc.tensor.matmul(sps[:], lhsT=ones[:], rhs=sq[:],
                                         start=True, stop=True)
                        srow = wpool.tile([1, TQ], f32, tag="srow", bufs=3,
                                          name=f"ksr{kh}")
                        nc.vector.tensor_copy(out=srow[:], in_=sps[:])
                        nc.sync.dma_start(kpk[kh:kh + 1, :], srow[:])
                        rrs_k.append(rr)
                    kln = wpool.tile([4, TQ], f32, tag="srow4", bufs=2,
                                     name="kln")
                    nc.scalar.activation(kln[:], kpk[:], Act.Ln, bias=EPS,
                                         scale=1.0 / HD)
                    krs = wpool.tile([4, TQ], f32, tag="srow4", bufs=2,
                                     name="krs")
                    nc.scalar.activation(krs[:], kln[:], Act.Exp, scale=-0.5)
                    for kh in range(4):
                        rowt = wpool.tile([1, TQ], f32, tag="srow", bufs=3,
                                          name=f"kurow{kh}")
                        nc.sync.dma_start(rowt[:], krs[kh:kh + 1, :])
                        sb = wpool.tile([128, TQ], f32, tag="sbcast", bufs=3)
                        nc.gpsimd.partition_broadcast(sb[:], rowt[:])
                        kt = qp.tile([128, TQ], bf16, tag="ktile", bufs=2,
                                     name="kt")
                        nc.vector.tensor_tensor(kt[:], rrs_k[kh][:], sb[:],
                                                Alu.mult)
                        nc.sync.dma_start(ckv_in[128 * kh:128 * (kh + 1), :],
                                          kt[:])

                    # ---- V heads (fp8 DR): token-major, then per-token
                    # pre-norm scale (with 1/64 folded in) ----
                    wvf = wslab2(wv, 0, TQ, NT, "wv", fp8)
                    for t in range(4):
                        vps = psA.tile([128, TQ], f32, tag="qkv", bufs=4,
                                       name=f"vps_{t}")
                        for ip in range(0, NT, 2):
                            nc.tensor.matmul(
                                vps[:],
                                lhsT=x8[:, ip:ip + 2, 128 * t:128 * (t + 1)],
                                rhs=wvf(ip, pair=True),
                                start=(ip == 0), stop=(ip == NT - 2),
                                perf_mode=DR)
                        vb = wpool.tile([128, TQ], bf16, tag="vb", bufs=2)
                        nc.vector.tensor_scalar_mul(vb[:], vps[:], s1c[:, t:t + 1])
                        nc.sync.dma_start(
                            ckv_in[512 + 128 * t:512 + 128 * (t + 1), :], vb[:])

                    # single combined K+V AllGather within the batch group
                    nc.gpsimd.collective_compute(
                        "AllGather", Alu.bypass,
                        replica_groups=[[0, 1, 2, 3], [4, 5, 6, 7]],
                        ins=[ckv_in[:]], outs=[ckv_out[:]])

                    # ---- Q heads (fp8 DR): project + rope + deferred q-norm
                    pending = None

                    def finish_q(pend):
                        hg, rrs, sqs = pend
                        qpk = wpool.tile([4, TQ], f32, tag="srow4", bufs=2,
                                         name=f"qpk{hg}")
                        for k in range(4):
                            sps = psA.tile([1, TQ], f32, tag="row", bufs=3)
                            nc.tensor.matmul(sps[:], lhsT=ones[:],
                                             rhs=sqs[k][:],
                                             start=True, stop=True)
                            srow = wpool.tile([1, TQ], f32, tag="srow",
                                              bufs=3, name=f"qsr{hg}_{k}")
                            nc.vector.tensor_copy(out=srow[:], in_=sps[:])
                            nc.sync.dma_start(qpk[k:k + 1, :], srow[:])
                        qln = wpool.tile([4, TQ], f32, tag="srow4", bufs=2,
                                         name=f"qln{hg}")
                        nc.scalar.activation(qln[:], qpk[:], Act.Ln,
                                             bias=HD * EPS, scale=1.0)
                        qrs = wpool.tile([4, TQ], f32, tag="srow4", bufs=2,
                                         name=f"qrs{hg}")
                        nc.scalar.activation(qrs[:], qln[:], Act.Exp,
                                             scale=-0.5)
                        for k in range(4):
                            rowt = wpool.tile([1, TQ], f32, tag="srow",
                                              bufs=3, name=f"qur{hg}_{k}")
                            nc.sync.dma_start(rowt[:], qrs[k:k + 1, :])
                            sb = wpool.tile([128, TQ], f32, tag="sbcast",
                                            bufs=3)
                            nc.gpsimd.partition_broadcast(sb[:], rowt[:])
                            nc.vector.tensor_tensor(qs_sb[:, 4 * hg + k],
                                                    rrs[k][:], sb[:], Alu.mult)

                    for hg in range(4):
                        qps = [psA.tile([128, TQ], f32, tag="qkv", bufs=4,
                                        name=f"qps{hg}_{_k}") for _k in range(4)]
                        wqf = wslab2(wq, NT * TQ * hg, TQ, NT, f"wq{hg}", fp8)
                        rrs = []
                        sqs = []
                        for k in range(4):
                            for ip in range(0, NT, 2):
                                nc.tensor.matmul(
                                    qps[k][:],
                                    lhsT=wqf(ip, 128 * k, 128 * (k + 1),
                                             pair=True),
                                    rhs=x8[:, ip:ip + 2, :],
                                    start=(ip == 0), stop=(ip == NT - 2),
                                    perf_mode=DR)
                            rrs.append(rope(qps[k], qp))
                            sqs.append(sumsq(rrs[k], qp))
                        if pending is not None:
                            finish_q(pending)
                        pending = (hg, rrs, sqs)
                    finish_q(pending)

                # ---- load gathered K/V (rank g block at rows 1024g; K kh at
                # +128kh, V col-block cb at +512+128cb); V cast to fp8 ----
                for gp in range(4):
                    nc.sync.dma_start(
                        k_sb[:, gp],
                        ckv_out[1024 * gp:1024 * gp + 512, :].rearrange(
                            "(kh p) t -> p kh t", p=128))
                    vbf = wpool.tile([128, 4, NKV * HD], bf16, tag="vbf",
                                     bufs=2)
                    nc.sync.dma_start(
                        vbf[:],
                        ckv_out[1024 * gp + 512:1024 * (gp + 1), :].rearrange(
                            "(cb p) f -> p cb f", p=128))
                    nc.vector.tensor_copy(out=v8_sb[:, gp], in_=vbf[:])
                qp_cm.__exit__(None, None, None)

                # ---- attention: fp8 p, DoubleRow y/den, DVE recip den ----
                ap_cm = tc.tile_pool(name="attntmp", bufs=1)
                ap = ap_cm.__enter__()
                mask_sb = ap.tile([128, 8, CH], bf16, tag="mask_sb")
                nc.sync.dma_start(mask_sb[:],
                                  mask.rearrange("p (d q) -> p d q", q=CH))
                with tc.tile_pool(name="psB", bufs=1, space="PSUM") as psB:
                    for kh in range(NKV):
                        den_kh = wpool.tile([8, CH], f32, tag="den_kh",
                                            bufs=2, name=f"dkh{kh}")
                        yraw_kh = ap.tile([128, 8, CH], bf16, tag="yraw",
                                          bufs=2, name=f"yr{kh}")
                        for j in range(4):
                            h = 4 * kh + j
                            for qt in range(2):
                                u = 2 * j + qt
                                nk = 8 * (qt + 1)
                                dlo = 8 * qt
                                y_ps = psB.tile([128, CH], f32, tag="y",
                                                bufs=2)
                                den_ps = psB.tile([16, CH], f32, tag="den",
                                                  bufs=2)
                                for grp in range(nk // 4):
                                    sc = psB.tile([128, 4, CH], f32, tag="sc",
                                                  bufs=2)
                                    for mi in range(4):
                                        m = 4 * grp + mi
                                        gp, cb = m % 4, m // 4
                                        nc.tensor.matmul(
                                            sc[:, mi],
                                            lhsT=k_sb[:, gp, kh,
                                                      128 * cb:128 * (cb + 1)],
                                            rhs=qs_sb[:, h,
                                                      CH * qt:CH * (qt + 1)],
                                            start=True, stop=True)
                                    p8 = ap.tile([128, 4, CH], fp8,
                                                 tag="p8", bufs=3)
                                    if 4 * grp >= dlo:
                                        # whole group is in the causal band
                                        pb = wpool.tile([128, 4, CH], bf16,
                                                        tag="pb", bufs=3)
                                        nc.scalar.activation(pb[:], sc[:],
                                                             Act.Exp,
                                                             bias=NLN16)
                                        for mi in range(4):
                                            m = 4 * grp + mi
                                            nc.vector.tensor_tensor(
                                                p8[:, mi], pb[:, mi],
                                                mask_sb[:, m - dlo], Alu.mult)
                                    else:
                                        nc.scalar.activation(p8[:], sc[:],
                                                             Act.Exp,
                                                             bias=NLN16)
                                    for mi in (0, 2):
                                        m = 4 * grp + mi
                                        gp, cb = m % 4, m // 4
                                        nc.tensor.matmul(
                                            y_ps[:],
                                            lhsT=v8_sb[:, gp:gp + 2, cb,
                                                       128 * kh:128 * (kh + 1)],
                                            rhs=p8[:, mi:mi + 2, :],
                                            start=(m == 0), stop=(m == nk - 2),
                                            perf_mode=DR)
                                        nc.tensor.matmul(
                                            den_ps[:],
                                            lhsT=ones2[:, :, 0:16],
                                            rhs=p8[:, mi:mi + 2, :],
                                            start=(m == 0), stop=(m == nk - 2),
                                            perf_mode=DR)
                                nc.vector.tensor_copy(out=yraw_kh[:, u],
                                                      in_=y_ps[:])
                                drow = wpool.tile([1, CH], f32, tag="srow",
                                                  bufs=3, name="drow")
                                nc.vector.tensor_copy(out=drow[:],
                                                      in_=den_ps[0:1, :])
                                nc.sync.dma_start(den_kh[u:u + 1, :], drow[:])
                        # batch recip for this kh group on the DVE; den_ps
                        # carried 1/(16*8) so rca = 8/den -> yT8 = 8*y
                        rcab = wpool.tile([8, CH], f32, tag="rcab", bufs=2,
                                          name=f"rc{kh}")
                        nc.vector.reciprocal_approx_fast(rcab[:], den_kh[:])
                        for j in range(4):
                            h = 4 * kh + j
                            for qt in range(2):
                                u = 2 * j + qt
                                rowt = wpool.tile([1, CH], f32, tag="srow",
                                                  bufs=3, name="durow")
                                nc.sync.dma_start(rowt[:], rcab[u:u + 1, :])
                                db = wpool.tile([128, CH], f32, tag="dbcast",
                                                bufs=2)
                                nc.gpsimd.partition_broadcast(db[:], rowt[:])
                                nc.vector.tensor_tensor(
                                    yT8[:, h, CH * qt:CH * (qt + 1)],
                                    yraw_kh[:, u], db[:], Alu.mult)

                ap_cm.__exit__(None, None, None)

                # ---- wo projection (fp8 DR) + residual (feature-major) ----
                with tc.tile_pool(name="psC", bufs=1, space="PSUM") as psC:
                    for n4 in range(4):
                        wof = wslab2(wo, NT * TQ * n4, TQ, NT, f"wo{n4}", fp8)
                        for k in range(4):
                            att_ps = psC.tile([128, TQ], f32, tag="att",
                                              bufs=4)
                            for hp in range(0, NH, 2):
                                nc.tensor.matmul(
                                    att_ps[:],
                                    lhsT=wof(hp, 128 * k, 128 * (k + 1),
                                             pair=True),
                                    rhs=yT8[:, hp:hp + 2, :],
                                    start=(hp == 0), stop=(hp == NH - 2),
                                    perf_mode=DR)
                            att_sb = wpool.tile([128, TQ], f32, tag="attsb",
                                                bufs=3)
                            # wo path carries 64 (w) * 8 (yT) = 512x
                            nc.scalar.activation(att_sb[:], att_ps[:],
                                                 Act.Copy, scale=1.0 / 512.0)
                            n = 4 * n4 + k
                            nc.vector.tensor_tensor(xmT[:, n], att_sb[:],
                                                    xin[:, n], Alu.add)
            # main pool closed (frees attention SBUF)

            # ---- MLP (bf16) ----
            with tc.tile_pool(name="mlp", bufs=1) as mlpp:
                h2T = mlpp.tile([128, NT, TQ], bf16, tag="h2T")
                a_sb = mlpp.tile([128, NF, TQ], bf16, tag="a_sb")

                with tc.tile_pool(name="psC2", bufs=1, space="PSUM") as psC2:
                    ssq2 = psC2.tile([1, TQ], f32, tag="row", bufs=2)
                    for i in range(NT):
                        xsq = wpool.tile([128, TQ], bf16, tag="xsq", bufs=6)
                        nc.vector.tensor_tensor(xsq[:], xmT[:, i], xmT[:, i],
                                                Alu.mult)
                        nc.tensor.matmul(ssq2[:], lhsT=ones[:], rhs=xsq[:],
                                         start=(i == 0), stop=(i == NT - 1))
                    s2b = norm_row(ssq2, 1.0 / C, EPS, TQ)
                    for i in range(NT):
                        nc.vector.tensor_tensor(h2T[:, i], xmT[:, i], s2b[:],
                                                Alu.mult)

                # fc + relu^2 (feature-major a)
                with tc.tile_pool(name="psD", bufs=1, space="PSUM") as psD:
                    for jc in range(16):
                        wfcf = wslab2(wfc, NT * TQ * jc, TQ, NT, f"wfc{jc}",
                                      bf16)
                        for jf in range(4):
                            f_ps = psD.tile([128, TQ], f32, tag="f", bufs=6)
                            for i in range(NT):
                                nc.tensor.matmul(
                                    f_ps[:],
                                    lhsT=wfcf(i, 128 * jf, 128 * (jf + 1)),
                                    rhs=h2T[:, i],
                                    start=(i == 0), stop=(i == NT - 1))
                            f = 4 * jc + jf
                            r_bf = wpool.tile([128, TQ], bf16, tag="r_bf")
                            nc.scalar.activation(r_bf[:], f_ps[:], Act.Relu)
                            nc.vector.tensor_tensor(a_sb[:, f], r_bf[:],
                                                    r_bf[:], Alu.mult)

                # proj: weight-stationary, feature-major output + residual
                with tc.tile_pool(name="psE", bufs=1, space="PSUM") as psE:
                    for n in range(16):
                        wpf = wslab2(wproj, NF * 128 * n, 128, NF, f"wp{n}",
                                     bf16)
                        o_ps = psE.tile([128, TQ], f32, tag="o", bufs=3)
                        for f in range(NF):
                            nc.tensor.matmul(o_ps[:], lhsT=wpf(f),
                                             rhs=a_sb[:, f, :],
                                             start=(f == 0),
                                             stop=(f == NF - 1))
                        ov = wpool.tile([128, TQ], f32, tag="ov", bufs=3)
                        nc.vector.tensor_tensor(ov[:], o_ps[:], xmT[:, n],
                                                Alu.add)
                        nc.sync.dma_start(out_fm[128 * n:128 * (n + 1), :],
                                          ov[:])

    nc.compile()
    return nc


def _prep_weights(wq, wk, wv, wo, w_fc, w_proj):
    import ml_dtypes
    bf = ml_dtypes.bfloat16
    f8 = ml_dtypes.float8_e4m3fn

    def tile_w(w, chunk, dtype, scale=1.0):
        # [R, F] -> [128, (F//chunk) * (R//128) * chunk]
        R, F = w.shape
        t = np.asarray(w, np.float32) * scale
        t = t.reshape(R // 128, 128, F // chunk, chunk)
        t = t.transpose(1, 2, 0, 3).reshape(128, -1)
        if dtype is f8:
            t = np.clip(t, -448.0, 448.0)
        return np.ascontiguousarray(t).astype(dtype)

    return {
        "wq": tile_w(wq, TQ, f8, SW),
        "wk": tile_w(wk, NKV * HD, f8, SW),
        "wv": tile_w(wv, NKV * HD, f8, SW),
        "wo": tile_w(wo, TQ, f8, SW),
        "wfc": tile_w(w_fc, TQ, bf),
        "wproj": tile_w(w_proj, 128, bf),
    }


def _make_in_maps(x, cos, sin, weights_b):
    import ml_dtypes
    bf = ml_dtypes.bfloat16
    f8 = ml_dtypes.float8_e4m3fn
    cosT = cos[0, :, 0, :].T  # [64, T]
    sinT = sin[0, :, 0, :].T
    kk = np.arange(128)
    qq = np.arange(CH)
    dd = np.arange(8)                           # band tile: d = dd//4, gp = dd%4
    in_maps = []
    for c in range(NCORES):
        b, g = divmod(c, 4)
        idx = 4 * np.arange(TQ) + g             # own token positions
        # key pos = 512*(2qt+d) + 4k + gp ; query pos = 1024qt + 4q + g
        off = 128 * (dd // 4) + (dd % 4 > g)
        msk = (qq[None, None, :] - kk[:, None, None]
               >= off[None, :, None]).astype(np.float32)
        xT = np.ascontiguousarray(x[b, idx, :].T)       # [C, TQ]
        xpt = xT.reshape(NT, 128, TQ).transpose(1, 0, 2).reshape(128, -1)
        m = {
            "xp": np.ascontiguousarray(xpt).astype(bf),
            "x8p": np.clip(np.ascontiguousarray(xpt), -448.0,
                           448.0).astype(f8),
            "csc": np.ascontiguousarray(
                np.concatenate([cosT[:, idx], cosT[:, idx]],
                               axis=0)).astype(bf),
            "css": np.ascontiguousarray(
                np.concatenate([sinT[:, idx], -sinT[:, idx]],
                               axis=0)).astype(bf),
            "mask": np.ascontiguousarray(msk.reshape(128, 8 * CH)).astype(bf),
        }
        m.update(weights_b)
        in_maps.append(m)
    return in_maps


def kernel(x, cos, sin, wq, wk, wv, wo, w_fc, w_proj):
    global _CACHE
    from concourse.bass_utils import run_bass_kernel_spmd

    x = np.asarray(x, np.float32)
    cos = np.asarray(cos, np.float32)
    sin = np.asarray(sin, np.float32)
    weights_b = _prep_weights(wq, wk, wv, wo, w_fc, w_proj)

    if _CACHE is None:
        _CACHE = _build()
    nc = _CACHE

    in_maps = _make_in_maps(x, cos, sin, weights_b)
    res = run_bass_kernel_spmd(nc, in_maps, list(range(NCORES)))
    out = np.empty((B, T, C), np.float32)
    for c in range(NCORES):
        b, g = divmod(c, 4)
        idx = 4 * np.arange(TQ) + g
        out[b, idx, :] = res.results[c]["out"].T
    return out
